# revision 30
# baseline (speedup 1.0000x reference)
"""Causal self-attention (B=2, T=2048, C=1024, H=16) on 8 trn2 NeuronCores.

Sharding: tensor-parallel over heads. Each core owns 2 heads (128 features):
  - qkv projection of the full sequence onto its 384 w_attn columns
  - causal attention for its 2 heads (both batches)
  - partial c_proj: y_local [4096,128] @ w_proj[rows] -> [4096,1024] partial
The 8 partial outputs are summed on the host (the "all-reduce after c_proj"),
plus b_proj.

Everything on-device runs in fp32r (TensorE fast fp32 mode, ~12 mantissa
bits, fp32 PSUM accumulation). End-to-end absmax-relative error vs the fp32
reference is ~8e-4 (measured via numpy simulation of fp32r rounding).

Layout trick: x is transposed on the host (x^T [1024, 4096]) so the qkv
projection consumes it directly as the moving operand; Q^T/K^T come out in
[feature, token] layout, which feeds the S^T = K^T-stationary attention
matmul with softmax denominators obtained from an extra ones-column in V.
"""

import sys

sys.path.insert(0, "/opt/trn_rl_repo")

import numpy as np

N_CORES = 8
B, T, C = 2, 2048, 1024
H, D = 16, 64
HPC = H // N_CORES            # heads per core
F = HPC * D                   # local feature width = 128
BT = B * T                    # 4096 tokens
TCH = 512                     # token chunk (moving-operand width)
NCH = BT // TCH               # 8 token chunks
KB = 128                      # kv block size
NQC = T // TCH                # 4 query chunks per batch

_COMPILED = {}


def _build():
    import concourse.bass as bass
    import concourse.mybir as mybir
    import concourse.tile as tile
    from concourse import bacc

    f32, f32r = mybir.dt.float32, mybir.dt.float32r
    Exp = mybir.ActivationFunctionType.Exp

    nc = bacc.Bacc("TRN2", target_bir_lowering=False, debug=False,
                   num_devices=N_CORES)

    xt = nc.dram_tensor("xt", [C, BT], f32r, kind="ExternalInput")
    wqkv = nc.dram_tensor("wqkv", [C // 128, 128, 3 * F], f32r,
                          kind="ExternalInput")
    bqkv = nc.dram_tensor("bqkv", [F, 3], f32, kind="ExternalInput")
    wp = nc.dram_tensor("wp", [F, C], f32r, kind="ExternalInput")
    tri = nc.dram_tensor("tri", [KB, KB], f32r, kind="ExternalInput")
    eye = nc.dram_tensor("eye", [128, 128], f32, kind="ExternalInput")
    ones = nc.dram_tensor("ones", [128, 64], f32r, kind="ExternalInput")
    out = nc.dram_tensor("out", [BT, C], f32, kind="ExternalOutput")

    with tile.TileContext(nc) as tc, \
         nc.allow_low_precision(reason="fp32r matmul pipeline, fp32 psum"):
        with tc.tile_pool(name="const", bufs=1) as cpool, \
             tc.tile_pool(name="seq", bufs=1) as seq, \
             tc.tile_pool(name="work", bufs=4) as work, \
             tc.tile_pool(name="psBig", bufs=2, space="PSUM") as psBig, \
             tc.tile_pool(name="psS", bufs=2, space="PSUM") as psS, \
             tc.tile_pool(name="psY", bufs=2, space="PSUM") as psY, \
             tc.tile_pool(name="dstage", bufs=4, space="DRAM") as dpool:

            # ---- resident constants ----
            w_sb = cpool.tile([128, C // 128, 3 * F], f32r)
            nc.sync.dma_start(w_sb[:], wqkv.rearrange("a p f -> p a f"))
            b_sb = cpool.tile([F, 3], f32)
            nc.sync.dma_start(b_sb[:], bqkv[:])
            wp_sb = cpool.tile([F, C], f32r)
            nc.gpsimd.dma_start(wp_sb[:], wp[:])
            tri_sb = cpool.tile([KB, KB], f32r)
            nc.gpsimd.dma_start(tri_sb[:], tri[:])
            eye_sb = cpool.tile([128, 128], f32)
            nc.gpsimd.dma_start(eye_sb[:], eye[:])
            ones_sb = cpool.tile([128, 64], f32r)
            nc.gpsimd.dma_start(ones_sb[:], ones[:])

            # ---- resident sequence tensors (per 512-token chunk tiles) ----
            qt_t = [seq.tile([F, TCH], f32r, tag=f"qt{t}", name=f"qt{t}") for t in range(NCH)]
            kt_t = [seq.tile([F, TCH], f32r, tag=f"kt{t}", name=f"kt{t}") for t in range(NCH)]
            # v65[:, i, 0:65] = [V_headA | 1], v65[:, i, 65:130] = [V_headB | 1]
            v65 = seq.tile([128, BT // KB, 130], f32r)
            nc.vector.tensor_copy(
                v65[:, :, 64::65],
                ones_sb[:, 0:64].rearrange("p (a b) -> p a b", b=2))
            yt_t = [seq.tile([F, TCH], f32r, tag=f"yt{t}", name=f"yt{t}") for t in range(NCH)]

            # ---- pre-zeroed diagonal P tiles (paired heads: [128,1024]) ----
            # halves: h0 cols [0:512), h1 cols [512:1024). For a diagonal
            # block with offset r, cols [0:128r) of each half are always
            # zero; zero them once, never rewrite.
            p_diag = {}
            for r in (1, 2, 3):
                for j in (0, 1):
                    pt = seq.tile([128, 2 * TCH], f32r, name=f"pdiag{r}_{j}")
                    pt3 = pt[:].rearrange("p (a q) -> p a q", a=2)
                    nc.vector.memset(pt3[:, :, 0:128 * r].bitcast(f32), 0.0)
                    p_diag[(r, j)] = pt

            # per-(b,bq) denominator tiles in wide layout [16, 64]:
            # rows 8h+p hold den[q = 64p + c] for head h
            den_w = {}
            for b in range(B):
                for bq in range(NQC):
                    den_w[(b, bq)] = seq.tile([16, 64], f32,
                                              name=f"denw{b}{bq}")

            def qkv_chunk_gen(t):
                """qkv projection + V transpose for one 512-token chunk.
                Part-outer: one PSUM accumulator live at a time."""
                xts = [work.tile([128, TCH], f32r, tag="xt", bufs=16,
                                 name=f"xts{t}_{i}") for i in range(8)]
                for cb in range(8):
                    nc.gpsimd.dma_start(
                        xts[cb][:],
                        xt[cb * 128:(cb + 1) * 128, t * TCH:(t + 1) * TCH])
                vt_tmp = None
                for part in range(3):
                    ps = psBig.tile([128, TCH], f32, tag="big",
                                    name=f"pqkv{t}_{part}")
                    for cb in range(8):
                        nc.tensor.matmul(
                            ps[:], w_sb[:, cb, part * F:(part + 1) * F],
                            xts[cb][:], start=(cb == 0), stop=(cb == 7))
                        if cb == 3:
                            yield
                    if part == 0:
                        nc.vector.tensor_scalar_add(qt_t[t][:], ps[:],
                                                    b_sb[:, 0:1])
                    elif part == 1:
                        nc.vector.tensor_scalar_add(kt_t[t][:], ps[:],
                                                    b_sb[:, 1:2])
                    else:
                        vt_tmp = work.tile([128, TCH], f32, tag="vt",
                                           name=f"vt{t}")
                        nc.vector.tensor_scalar_add(vt_tmp[:], ps[:],
                                                    b_sb[:, 2:3])
                    yield
                ptr = psS.tile([128, TCH], f32, tag="s", name=f"ptr{t}")
                for i in range(4):
                    nc.tensor.transpose(ptr[:, i * 128:(i + 1) * 128],
                                        vt_tmp[:, i * 128:(i + 1) * 128],
                                        eye_sb[:])
                    if i == 1:
                        yield
                ptr3 = ptr[:].rearrange("p (a k) -> p a k", k=128)
                t4 = t * 4
                nc.vector.tensor_copy(v65[:, t4:t4 + 4, 0:64],
                                      ptr3[:, :, 0:64])
                nc.vector.tensor_copy(v65[:, t4:t4 + 4, 65:129],
                                      ptr3[:, :, 64:128])
                yield

            def norm_proj_gen(b, bq):
                """Per-chunk softmax normalization + projection."""
                qchunk = b * NQC + bq
                rec_w = work.tile([16, 64], f32r, tag="rec",
                                  name=f"rec{b}{bq}")
                nc.vector.reciprocal(rec_w[:], den_w[(b, bq)][:])
                for h in range(HPC):
                    hs = h * 64
                    rr = dpool.tile([1, TCH], f32r, tag="rr",
                                    name=f"rr{b}{bq}{h}")
                    nc.gpsimd.dma_start(
                        rr[:].rearrange("o (p c) -> (o p) c", c=64),
                        rec_w[8 * h:8 * h + 8, :])
                    rst = work.tile([1, TCH], f32r, tag="rst",
                                    name=f"rst{b}{bq}{h}")
                    nc.gpsimd.dma_start(rst[:], rr[:])
                    bcast = work.tile([128, TCH], f32r, tag="bcast",
                                      name=f"bcast{b}{bq}{h}")
                    nc.gpsimd.partition_broadcast(bcast[:], rst[:])
                    nc.vector.tensor_mul(yt_t[qchunk][hs:hs + 64, :],
                                         yt_t[qchunk][hs:hs + 64, :],
                                         bcast[hs:hs + 64, :])
                yield
                for ic in range(4):
                    tt = qchunk * 4 + ic
                    for cc in range(2):
                        pj = psBig.tile([128, TCH], f32, tag="big",
                                        name=f"pj{tt}_{cc}")
                        nc.tensor.matmul(
                            pj[:],
                            yt_t[qchunk][:, ic * 128:(ic + 1) * 128],
                            wp_sb[:, cc * TCH:(cc + 1) * TCH],
                            start=True, stop=True)
                        ost = work.tile([128, TCH], f32, tag="ost",
                                        name=f"ost{tt}_{cc}")
                        if (ic + cc) % 2 == 0:
                            nc.scalar.copy(ost[:], pj[:])
                        else:
                            nc.vector.tensor_copy(ost[:], pj[:])
                        nc.sync.dma_start(
                            out[tt * 128:(tt + 1) * 128,
                                cc * TCH:(cc + 1) * TCH], ost[:])
                        yield

            class Filler:
                def __init__(self):
                    self.gens = []

                def add(self, g):
                    self.gens.append(g)

                def step(self):
                    while self.gens:
                        try:
                            next(self.gens[0])
                            return
                        except StopIteration:
                            self.gens.pop(0)

                def drain(self):
                    while self.gens:
                        for _ in self.gens.pop(0):
                            pass

            def attn_pair(b, bq, bk, use_idx):
                """S for both heads into one [128,1024] psum tile + one exp.
                Returns the P tile (halves = heads)."""
                qchunk = b * NQC + bq
                kchunk = b * NQC + bk // 4
                kcol = (bk % 4) * 128
                s_ps = psS.tile([128, 2 * TCH], f32, tag="s",
                                name=f"s{b}{bq}{bk}")
                r = bk - 4 * bq
                # masked q-columns [0:128r) can be skipped entirely when the
                # remaining width stays >= 256 (fp32r full-rate threshold)
                trim = 128 * r if r in (1, 2) else 0
                for h in range(HPC):
                    hs = h * 64
                    nc.tensor.matmul(
                        s_ps[:, h * TCH + trim:(h + 1) * TCH],
                        kt_t[kchunk][hs:hs + 64, kcol:kcol + 128],
                        qt_t[qchunk][hs:hs + 64, trim:],
                        start=True, stop=True)
                if r < 0:
                    p_t = work.tile([128, 2 * TCH], f32r, tag="p", bufs=4,
                                    name=f"p{b}{bq}{bk}")
                    nc.scalar.activation(p_t[:], s_ps[:], Exp)
                    return p_t
                if r == 0:
                    p_t = work.tile([128, 2 * TCH], f32r, tag="p", bufs=4,
                                    name=f"p{b}{bq}{bk}")
                    nc.scalar.activation(p_t[:], s_ps[:], Exp)
                else:
                    p_t = p_diag[(r, use_idx % 2)]
                    s3 = s_ps[:].rearrange("p (a q) -> p a q", a=2)
                    p3 = p_t[:].rearrange("p (a q) -> p a q", a=2)
                    nc.scalar.activation(p3[:, :, 128 * r:],
                                         s3[:, :, 128 * r:], Exp)
                for h in range(HPC):
                    c0 = h * TCH + 128 * r
                    nc.vector.tensor_mul(p_t[:, c0:c0 + 128],
                                         p_t[:, c0:c0 + 128], tri_sb[:])
                return p_t

            def attention_chunk(b, bq, fl):
                qchunk = b * NQC + bq
                nblk = 4 * bq + 4
                yt_ps = [psY.tile([65, TCH], f32, tag="yt",
                                  name=f"ytps{b}{bq}{h}")
                         for h in range(HPC)]
                pend = None

                def emit_pv(bk, p_t, stop):
                    vti = b * (T // KB) + bk
                    r = bk - 4 * bq
                    trim = 128 * r if r in (1, 2) else 0
                    for h in range(HPC):
                        nc.tensor.matmul(
                            yt_ps[h][:, trim:], v65[:, vti, 65 * h:65 * h + 65],
                            p_t[:, h * TCH + trim:(h + 1) * TCH],
                            start=(bk == 0), stop=stop)

                for bk in range(nblk):
                    p_t = attn_pair(b, bq, bk, bq)
                    if pend is not None:
                        emit_pv(pend[0], pend[1], stop=False)
                    pend = (bk, p_t)
                    fl.step()
                emit_pv(pend[0], pend[1], stop=True)
                for h in range(HPC):
                    hs = h * 64
                    nc.vector.tensor_copy(yt_t[qchunk][hs:hs + 64, :],
                                          yt_ps[h][0:64, :])
                    dst = work.tile([1, TCH], f32, tag="dst",
                                    name=f"dst{b}{bq}{h}")
                    nc.vector.tensor_copy(dst[:], yt_ps[h][64:65, :])
                    dd = dpool.tile([1, TCH], f32, tag="dd",
                                    name=f"dd{b}{bq}{h}")
                    nc.gpsimd.dma_start(dd[:], dst[:])
                    nc.gpsimd.dma_start(
                        den_w[(b, bq)][8 * h:8 * h + 8, :],
                        dd[:].rearrange("o (p c) -> (o p) c", c=64))
                fl.step()

            def chain(*gens):
                for g in gens:
                    yield from g

            # ---- schedule ----
            for t in range(NQC):
                for _ in qkv_chunk_gen(t):
                    pass
            fl = Filler()
            fl.add(chain(*[qkv_chunk_gen(t) for t in range(NQC, NCH)]))
            pending_np = []
            for b in range(B):
                for bq in range(NQC):
                    attention_chunk(b, bq, fl)
                    if pending_np:
                        fl.add(pending_np.pop(0))
                    pending_np.append(norm_proj_gen(b, bq))
            for g in pending_np:
                fl.add(g)
            fl.drain()
    nc.compile()
    return nc


def _get_nc():
    if "nc" not in _COMPILED:
        _COMPILED["nc"] = _build()
    return _COMPILED["nc"]


def _prep_in_maps(x, w_attn, b_attn, w_proj):
    x = np.asarray(x, np.float32)
    w_attn = np.asarray(w_attn, np.float32)
    b_attn = np.asarray(b_attn, np.float32)
    w_proj = np.asarray(w_proj, np.float32)

    scale = np.float32(1.0 / np.sqrt(D))
    xt = np.ascontiguousarray(x.reshape(BT, C).T)          # [C, BT]
    # tri[kv, j] = 1 when j >= kv (upper triangular incl diagonal)
    tri = np.ascontiguousarray(np.triu(np.ones((KB, KB), np.float32)))
    eye = np.eye(128, dtype=np.float32)
    ones = np.ones((128, 64), np.float32)

    in_maps = []
    for c in range(N_CORES):
        cols = slice(c * F, (c + 1) * F)
        wq = w_attn[:, cols] * scale
        wk = w_attn[:, C + c * F:C + (c + 1) * F]
        wv = w_attn[:, 2 * C + c * F:2 * C + (c + 1) * F]
        wqkv = np.ascontiguousarray(
            np.concatenate([wq, wk, wv], axis=1).reshape(C // 128, 128, 3 * F))
        bq = b_attn[c * F:(c + 1) * F] * scale
        bk = b_attn[C + c * F:C + (c + 1) * F]
        bv = b_attn[2 * C + c * F:2 * C + (c + 1) * F]
        bqkv = np.ascontiguousarray(np.stack([bq, bk, bv], axis=1))
        wp = np.ascontiguousarray(w_proj[c * F:(c + 1) * F, :])
        in_maps.append({
            "xt": xt, "wqkv": wqkv, "bqkv": bqkv, "wp": wp,
            "tri": tri, "eye": eye, "ones": ones,
        })
    return in_maps


def _run(inputs, trace=False):
    from concourse.bass_utils import run_bass_kernel_spmd

    nc = _get_nc()
    in_maps = _prep_in_maps(inputs["x"], inputs["w_attn"], inputs["b_attn"],
                            inputs["w_proj"])
    res = run_bass_kernel_spmd(nc, in_maps, list(range(N_CORES)), trace=trace)
    b_proj = np.asarray(inputs["b_proj"], np.float32)
    acc = np.zeros((BT, C), np.float64)
    for c in range(N_CORES):
        acc += res.results[c]["out"]
    y = (acc + b_proj).astype(np.float32).reshape(B, T, C)
    return y, res


def kernel(**inputs):
    y, _ = _run(inputs, trace=False)
    return y


# revision 31
# speedup vs baseline: 1.0485x; 1.0485x over previous
"""Causal self-attention (B=2, T=2048, C=1024, H=16) on 8 trn2 NeuronCores.

Sharding: tensor-parallel over heads. Each core owns 2 heads (128 features):
  - qkv projection of the full sequence onto its 384 w_attn columns
  - causal attention for its 2 heads (both batches)
  - partial c_proj: y_local [4096,128] @ w_proj[rows] -> [4096,1024] partial
The 8 partial outputs are summed on the host (the "all-reduce after c_proj"),
plus b_proj.

Everything on-device runs in fp32r (TensorE fast fp32 mode, ~12 mantissa
bits, fp32 PSUM accumulation). End-to-end absmax-relative error vs the fp32
reference is ~8e-4 (measured via numpy simulation of fp32r rounding).

Layout trick: x is transposed on the host (x^T [1024, 4096]) so the qkv
projection consumes it directly as the moving operand; Q^T/K^T come out in
[feature, token] layout, which feeds the S^T = K^T-stationary attention
matmul with softmax denominators obtained from an extra ones-column in V.
"""

import sys

sys.path.insert(0, "/opt/trn_rl_repo")

import numpy as np

N_CORES = 8
B, T, C = 2, 2048, 1024
H, D = 16, 64
HPC = H // N_CORES            # heads per core
F = HPC * D                   # local feature width = 128
BT = B * T                    # 4096 tokens
TCH = 512                     # token chunk (moving-operand width)
NCH = BT // TCH               # 8 token chunks
KB = 128                      # kv block size
NQC = T // TCH                # 4 query chunks per batch

_COMPILED = {}


def _build():
    import concourse.bass as bass
    import concourse.mybir as mybir
    import concourse.tile as tile
    from concourse import bacc

    f32, f32r = mybir.dt.float32, mybir.dt.float32r
    Exp = mybir.ActivationFunctionType.Exp

    nc = bacc.Bacc("TRN2", target_bir_lowering=False, debug=False,
                   num_devices=N_CORES)

    xt = nc.dram_tensor("xt", [C, BT], f32r, kind="ExternalInput")
    wqkv = nc.dram_tensor("wqkv", [C // 128, 128, 3 * F], f32r,
                          kind="ExternalInput")
    bqkv = nc.dram_tensor("bqkv", [F, 3], f32, kind="ExternalInput")
    wp = nc.dram_tensor("wp", [F, C], f32r, kind="ExternalInput")
    tri = nc.dram_tensor("tri", [KB, KB], f32r, kind="ExternalInput")
    eye = nc.dram_tensor("eye", [128, 128], f32, kind="ExternalInput")
    ones = nc.dram_tensor("ones", [128, 64], f32r, kind="ExternalInput")
    out = nc.dram_tensor("out", [BT, C], f32, kind="ExternalOutput")

    with tile.TileContext(nc) as tc, \
         nc.allow_low_precision(reason="fp32r matmul pipeline, fp32 psum"):
        with tc.tile_pool(name="const", bufs=1) as cpool, \
             tc.tile_pool(name="seq", bufs=1) as seq, \
             tc.tile_pool(name="work", bufs=4) as work, \
             tc.tile_pool(name="psBig", bufs=2, space="PSUM") as psBig, \
             tc.tile_pool(name="psS", bufs=2, space="PSUM") as psS, \
             tc.tile_pool(name="psY", bufs=2, space="PSUM") as psY, \
             tc.tile_pool(name="dstage", bufs=4, space="DRAM") as dpool:

            # ---- resident constants ----
            w_sb = cpool.tile([128, C // 128, 3 * F], f32r)
            nc.sync.dma_start(w_sb[:], wqkv.rearrange("a p f -> p a f"))
            b_sb = cpool.tile([F, 3], f32)
            nc.sync.dma_start(b_sb[:], bqkv[:])
            wp_sb = cpool.tile([F, C], f32r)
            nc.gpsimd.dma_start(wp_sb[:], wp[:])
            tri_sb = cpool.tile([KB, KB], f32r)
            nc.gpsimd.dma_start(tri_sb[:], tri[:])
            eye_sb = cpool.tile([128, 128], f32)
            nc.gpsimd.dma_start(eye_sb[:], eye[:])
            ones_sb = cpool.tile([128, 64], f32r)
            nc.gpsimd.dma_start(ones_sb[:], ones[:])

            # ---- resident sequence tensors (per 512-token chunk tiles) ----
            qt_t = [seq.tile([F, TCH], f32r, tag=f"qt{t}", name=f"qt{t}") for t in range(NCH)]
            kt_t = [seq.tile([F, TCH], f32r, tag=f"kt{t}", name=f"kt{t}") for t in range(NCH)]
            # v65[:, i, 0:65] = [V_headA | 1], v65[:, i, 65:130] = [V_headB | 1]
            v65 = seq.tile([128, BT // KB, 130], f32r)
            nc.vector.tensor_copy(
                v65[:, :, 64::65],
                ones_sb[:, 0:64].rearrange("p (a b) -> p a b", b=2))
            yt_t = [seq.tile([F, TCH], f32r, tag=f"yt{t}", name=f"yt{t}") for t in range(NCH)]

            # ---- pre-zeroed diagonal P tiles (paired heads: [128,1024]) ----
            # halves: h0 cols [0:512), h1 cols [512:1024). For a diagonal
            # block with offset r, cols [0:128r) of each half are always
            # zero; zero them once, never rewrite.
            p_diag = {}
            for r in (1, 2, 3):
                for j in (0, 1):
                    pt = seq.tile([128, 2 * TCH], f32r, name=f"pdiag{r}_{j}")
                    pt3 = pt[:].rearrange("p (a q) -> p a q", a=2)
                    nc.vector.memset(pt3[:, :, 0:128 * r].bitcast(f32), 0.0)
                    p_diag[(r, j)] = pt

            # per-(b,bq) denominator tiles in wide layout [16, 64]:
            # rows 8h+p hold den[q = 64p + c] for head h
            den_w = {}
            for b in range(B):
                for bq in range(NQC):
                    den_w[(b, bq)] = seq.tile([16, 64], f32,
                                              name=f"denw{b}{bq}")

            def qkv_chunk_gen(t):
                """qkv projection + V transpose for one 512-token chunk.
                Part-outer: one PSUM accumulator live at a time."""
                xts = [work.tile([128, TCH], f32r, tag="xt", bufs=16,
                                 name=f"xts{t}_{i}") for i in range(8)]
                for cb in range(8):
                    nc.gpsimd.dma_start(
                        xts[cb][:],
                        xt[cb * 128:(cb + 1) * 128, t * TCH:(t + 1) * TCH])
                vt_tmp = None
                for part in range(3):
                    ps = psBig.tile([128, TCH], f32, tag="big",
                                    name=f"pqkv{t}_{part}")
                    for cb in range(8):
                        nc.tensor.matmul(
                            ps[:], w_sb[:, cb, part * F:(part + 1) * F],
                            xts[cb][:], start=(cb == 0), stop=(cb == 7))
                        if cb == 3:
                            yield
                    if part == 0:
                        nc.vector.tensor_scalar_add(qt_t[t][:], ps[:],
                                                    b_sb[:, 0:1])
                    elif part == 1:
                        nc.vector.tensor_scalar_add(kt_t[t][:], ps[:],
                                                    b_sb[:, 1:2])
                    else:
                        vt_tmp = work.tile([128, TCH], f32, tag="vt",
                                           name=f"vt{t}")
                        nc.vector.tensor_scalar_add(vt_tmp[:], ps[:],
                                                    b_sb[:, 2:3])
                    yield
                ptr = psS.tile([128, TCH], f32, tag="s", name=f"ptr{t}")
                for i in range(4):
                    nc.tensor.transpose(ptr[:, i * 128:(i + 1) * 128],
                                        vt_tmp[:, i * 128:(i + 1) * 128],
                                        eye_sb[:])
                    if i == 1:
                        yield
                ptr3 = ptr[:].rearrange("p (a k) -> p a k", k=128)
                t4 = t * 4
                nc.vector.tensor_copy(v65[:, t4:t4 + 4, 0:64],
                                      ptr3[:, :, 0:64])
                nc.vector.tensor_copy(v65[:, t4:t4 + 4, 65:129],
                                      ptr3[:, :, 64:128])
                yield

            def norm_proj_gen(b, bq):
                """Per-chunk softmax normalization + projection."""
                qchunk = b * NQC + bq
                rec_w = work.tile([16, 64], f32r, tag="rec",
                                  name=f"rec{b}{bq}")
                nc.vector.reciprocal(rec_w[:], den_w[(b, bq)][:])
                for h in range(HPC):
                    hs = h * 64
                    rr = dpool.tile([1, TCH], f32r, tag="rr",
                                    name=f"rr{b}{bq}{h}")
                    nc.gpsimd.dma_start(
                        rr[:].rearrange("o (p c) -> (o p) c", c=64),
                        rec_w[8 * h:8 * h + 8, :])
                    rst = work.tile([1, TCH], f32r, tag="rst",
                                    name=f"rst{b}{bq}{h}")
                    nc.gpsimd.dma_start(rst[:], rr[:])
                    bcast = work.tile([128, TCH], f32r, tag="bcast",
                                      name=f"bcast{b}{bq}{h}")
                    nc.gpsimd.partition_broadcast(bcast[:], rst[:])
                    nc.vector.tensor_mul(yt_t[qchunk][hs:hs + 64, :],
                                         yt_t[qchunk][hs:hs + 64, :],
                                         bcast[hs:hs + 64, :])
                yield
                for ic in range(4):
                    tt = qchunk * 4 + ic
                    for cc in range(2):
                        pj = psBig.tile([128, TCH], f32, tag="big",
                                        name=f"pj{tt}_{cc}")
                        nc.tensor.matmul(
                            pj[:],
                            yt_t[qchunk][:, ic * 128:(ic + 1) * 128],
                            wp_sb[:, cc * TCH:(cc + 1) * TCH],
                            start=True, stop=True)
                        ost = work.tile([128, TCH], f32, tag="ost",
                                        name=f"ost{tt}_{cc}")
                        if (ic + cc) % 2 == 0:
                            nc.scalar.copy(ost[:], pj[:])
                        else:
                            nc.vector.tensor_copy(ost[:], pj[:])
                        nc.sync.dma_start(
                            out[tt * 128:(tt + 1) * 128,
                                cc * TCH:(cc + 1) * TCH], ost[:])
                        yield

            class Filler:
                def __init__(self):
                    self.gens = []

                def add(self, g):
                    self.gens.append(g)

                def step(self):
                    while self.gens:
                        try:
                            next(self.gens[0])
                            return
                        except StopIteration:
                            self.gens.pop(0)

                def drain(self):
                    while self.gens:
                        for _ in self.gens.pop(0):
                            pass

            def attn_pair(b, bq, bk, use_idx):
                """S for both heads into one [128,1024] psum tile + one exp.
                Returns the P tile (halves = heads)."""
                qchunk = b * NQC + bq
                kchunk = b * NQC + bk // 4
                kcol = (bk % 4) * 128
                s_ps = psS.tile([128, 2 * TCH], f32, tag="s",
                                name=f"s{b}{bq}{bk}")
                r = bk - 4 * bq
                # masked q-columns [0:128r) can be skipped entirely when the
                # remaining width stays >= 256 (fp32r full-rate threshold)
                trim = 128 * r if r in (1, 2) else 0
                for h in range(HPC):
                    hs = h * 64
                    nc.tensor.matmul(
                        s_ps[:, h * TCH + trim:(h + 1) * TCH],
                        kt_t[kchunk][hs:hs + 64, kcol:kcol + 128],
                        qt_t[qchunk][hs:hs + 64, trim:],
                        start=True, stop=True)
                if r < 0:
                    p_t = work.tile([128, 2 * TCH], f32r, tag="p", bufs=4,
                                    name=f"p{b}{bq}{bk}")
                    nc.scalar.activation(p_t[:], s_ps[:], Exp)
                    return p_t
                if r == 0:
                    p_t = work.tile([128, 2 * TCH], f32r, tag="p", bufs=4,
                                    name=f"p{b}{bq}{bk}")
                    nc.scalar.activation(p_t[:], s_ps[:], Exp)
                else:
                    p_t = p_diag[(r, use_idx % 2)]
                    s3 = s_ps[:].rearrange("p (a q) -> p a q", a=2)
                    p3 = p_t[:].rearrange("p (a q) -> p a q", a=2)
                    nc.scalar.activation(p3[:, :, 128 * r:],
                                         s3[:, :, 128 * r:], Exp)
                for h in range(HPC):
                    c0 = h * TCH + 128 * r
                    nc.vector.tensor_mul(p_t[:, c0:c0 + 128],
                                         p_t[:, c0:c0 + 128], tri_sb[:])
                return p_t

            def attention_chunk(b, bq, fl):
                qchunk = b * NQC + bq
                nblk = 4 * bq + 4
                yt_ps = [psY.tile([65, TCH], f32, tag="yt",
                                  name=f"ytps{b}{bq}{h}")
                         for h in range(HPC)]
                pend = None

                def emit_pv(bk, p_t, stop):
                    vti = b * (T // KB) + bk
                    r = bk - 4 * bq
                    trim = 128 * r if r in (1, 2) else 0
                    for h in range(HPC):
                        nc.tensor.matmul(
                            yt_ps[h][:, trim:], v65[:, vti, 65 * h:65 * h + 65],
                            p_t[:, h * TCH + trim:(h + 1) * TCH],
                            start=(bk == 0), stop=stop)

                for bk in range(nblk):
                    p_t = attn_pair(b, bq, bk, bq)
                    if pend is not None:
                        emit_pv(pend[0], pend[1], stop=False)
                    pend = (bk, p_t)
                    fl.step()
                emit_pv(pend[0], pend[1], stop=True)
                for h in range(HPC):
                    hs = h * 64
                    nc.vector.tensor_copy(yt_t[qchunk][hs:hs + 64, :],
                                          yt_ps[h][0:64, :])
                    dst = work.tile([1, TCH], f32, tag="dst",
                                    name=f"dst{b}{bq}{h}")
                    nc.vector.tensor_copy(dst[:], yt_ps[h][64:65, :])
                    dd = dpool.tile([1, TCH], f32, tag="dd",
                                    name=f"dd{b}{bq}{h}")
                    nc.gpsimd.dma_start(dd[:], dst[:])
                    nc.gpsimd.dma_start(
                        den_w[(b, bq)][8 * h:8 * h + 8, :],
                        dd[:].rearrange("o (p c) -> (o p) c", c=64))
                fl.step()

            def chain(*gens):
                for g in gens:
                    yield from g

            # ---- schedule ----
            for t in range(NQC):
                for _ in qkv_chunk_gen(t):
                    pass
            fl = Filler()
            fl.add(chain(*[qkv_chunk_gen(t) for t in range(NQC, NCH)]))
            pending_np = []
            for b in range(B):
                for bq in range(NQC):
                    if b == B - 1 and bq == NQC - 1:
                        # last chunk: make all pending norm+proj work
                        # available as filler so only the final chunk's
                        # chain remains at the tail
                        while pending_np:
                            fl.add(pending_np.pop(0))
                    attention_chunk(b, bq, fl)
                    if pending_np:
                        fl.add(pending_np.pop(0))
                    pending_np.append(norm_proj_gen(b, bq))
            for g in pending_np:
                fl.add(g)
            fl.drain()
    nc.compile()
    return nc


def _get_nc():
    if "nc" not in _COMPILED:
        _COMPILED["nc"] = _build()
    return _COMPILED["nc"]


def _prep_in_maps(x, w_attn, b_attn, w_proj):
    x = np.asarray(x, np.float32)
    w_attn = np.asarray(w_attn, np.float32)
    b_attn = np.asarray(b_attn, np.float32)
    w_proj = np.asarray(w_proj, np.float32)

    scale = np.float32(1.0 / np.sqrt(D))
    xt = np.ascontiguousarray(x.reshape(BT, C).T)          # [C, BT]
    # tri[kv, j] = 1 when j >= kv (upper triangular incl diagonal)
    tri = np.ascontiguousarray(np.triu(np.ones((KB, KB), np.float32)))
    eye = np.eye(128, dtype=np.float32)
    ones = np.ones((128, 64), np.float32)

    in_maps = []
    for c in range(N_CORES):
        cols = slice(c * F, (c + 1) * F)
        wq = w_attn[:, cols] * scale
        wk = w_attn[:, C + c * F:C + (c + 1) * F]
        wv = w_attn[:, 2 * C + c * F:2 * C + (c + 1) * F]
        wqkv = np.ascontiguousarray(
            np.concatenate([wq, wk, wv], axis=1).reshape(C // 128, 128, 3 * F))
        bq = b_attn[c * F:(c + 1) * F] * scale
        bk = b_attn[C + c * F:C + (c + 1) * F]
        bv = b_attn[2 * C + c * F:2 * C + (c + 1) * F]
        bqkv = np.ascontiguousarray(np.stack([bq, bk, bv], axis=1))
        wp = np.ascontiguousarray(w_proj[c * F:(c + 1) * F, :])
        in_maps.append({
            "xt": xt, "wqkv": wqkv, "bqkv": bqkv, "wp": wp,
            "tri": tri, "eye": eye, "ones": ones,
        })
    return in_maps


def _run(inputs, trace=False):
    from concourse.bass_utils import run_bass_kernel_spmd

    nc = _get_nc()
    in_maps = _prep_in_maps(inputs["x"], inputs["w_attn"], inputs["b_attn"],
                            inputs["w_proj"])
    res = run_bass_kernel_spmd(nc, in_maps, list(range(N_CORES)), trace=trace)
    b_proj = np.asarray(inputs["b_proj"], np.float32)
    acc = np.zeros((BT, C), np.float64)
    for c in range(N_CORES):
        acc += res.results[c]["out"]
    y = (acc + b_proj).astype(np.float32).reshape(B, T, C)
    return y, res


def kernel(**inputs):
    y, _ = _run(inputs, trace=False)
    return y


# revision 32
# speedup vs baseline: 1.0820x; 1.0320x over previous
"""Causal self-attention (B=2, T=2048, C=1024, H=16) on 8 trn2 NeuronCores.

Sharding: tensor-parallel over heads. Each core owns 2 heads (128 features):
  - qkv projection of the full sequence onto its 384 w_attn columns
  - causal attention for its 2 heads (both batches)
  - partial c_proj: y_local [4096,128] @ w_proj[rows] -> [4096,1024] partial
The 8 partial outputs are summed on the host (the "all-reduce after c_proj"),
plus b_proj.

Everything on-device runs in fp32r (TensorE fast fp32 mode, ~12 mantissa
bits, fp32 PSUM accumulation). End-to-end absmax-relative error vs the fp32
reference is ~8e-4 (measured via numpy simulation of fp32r rounding).

Layout trick: x is transposed on the host (x^T [1024, 4096]) so the qkv
projection consumes it directly as the moving operand; Q^T/K^T come out in
[feature, token] layout, which feeds the S^T = K^T-stationary attention
matmul with softmax denominators obtained from an extra ones-column in V.
"""

import sys

sys.path.insert(0, "/opt/trn_rl_repo")

import numpy as np

N_CORES = 8
B, T, C = 2, 2048, 1024
H, D = 16, 64
HPC = H // N_CORES            # heads per core
F = HPC * D                   # local feature width = 128
BT = B * T                    # 4096 tokens
TCH = 512                     # token chunk (moving-operand width)
NCH = BT // TCH               # 8 token chunks
KB = 128                      # kv block size
NQC = T // TCH                # 4 query chunks per batch

_COMPILED = {}


def _build():
    import concourse.bass as bass
    import concourse.mybir as mybir
    import concourse.tile as tile
    from concourse import bacc

    f32, f32r = mybir.dt.float32, mybir.dt.float32r
    Exp = mybir.ActivationFunctionType.Exp

    nc = bacc.Bacc("TRN2", target_bir_lowering=False, debug=False,
                   num_devices=N_CORES)

    xt = nc.dram_tensor("xt", [C, BT], f32r, kind="ExternalInput")
    wqkv = nc.dram_tensor("wqkv", [C // 128, 128, 3 * F], f32r,
                          kind="ExternalInput")
    bqkv = nc.dram_tensor("bqkv", [F, 3], f32, kind="ExternalInput")
    wp = nc.dram_tensor("wp", [F, C], f32r, kind="ExternalInput")
    tri = nc.dram_tensor("tri", [KB, KB], f32r, kind="ExternalInput")
    eye = nc.dram_tensor("eye", [128, 128], f32, kind="ExternalInput")
    ones = nc.dram_tensor("ones", [128, 64], f32r, kind="ExternalInput")
    out = nc.dram_tensor("out", [BT, C], f32, kind="ExternalOutput")

    with tile.TileContext(nc) as tc, \
         nc.allow_low_precision(reason="fp32r matmul pipeline, fp32 psum"):
        with tc.tile_pool(name="const", bufs=1) as cpool, \
             tc.tile_pool(name="seq", bufs=1) as seq, \
             tc.tile_pool(name="work", bufs=4) as work, \
             tc.tile_pool(name="psBig", bufs=2, space="PSUM") as psBig, \
             tc.tile_pool(name="psS", bufs=2, space="PSUM") as psS, \
             tc.tile_pool(name="psY", bufs=2, space="PSUM") as psY, \
             tc.tile_pool(name="dstage", bufs=4, space="DRAM") as dpool:

            # ---- resident constants ----
            w_sb = cpool.tile([128, C // 128, 3 * F], f32r)
            nc.sync.dma_start(w_sb[:], wqkv.rearrange("a p f -> p a f"))
            b_sb = cpool.tile([F, 3], f32)
            nc.sync.dma_start(b_sb[:], bqkv[:])
            wp_sb = cpool.tile([F, C], f32r)
            nc.gpsimd.dma_start(wp_sb[:], wp[:])
            tri_sb = cpool.tile([KB, KB], f32r)
            nc.gpsimd.dma_start(tri_sb[:], tri[:])
            eye_sb = cpool.tile([128, 128], f32)
            nc.gpsimd.dma_start(eye_sb[:], eye[:])
            ones_sb = cpool.tile([128, 64], f32r)
            nc.gpsimd.dma_start(ones_sb[:], ones[:])

            # ---- resident sequence tensors (per 512-token chunk tiles) ----
            qt_t = [seq.tile([F, TCH], f32r, tag=f"qt{t}", name=f"qt{t}") for t in range(NCH)]
            kt_t = [seq.tile([F, TCH], f32r, tag=f"kt{t}", name=f"kt{t}") for t in range(NCH)]
            # v65[:, i, 0:65] = [V_headA | 1], v65[:, i, 65:130] = [V_headB | 1]
            v65 = seq.tile([128, BT // KB, 130], f32r)
            nc.vector.tensor_copy(
                v65[:, :, 64::65],
                ones_sb[:, 0:64].rearrange("p (a b) -> p a b", b=2))
            yt_t = [seq.tile([F, TCH], f32r, tag=f"yt{t}", name=f"yt{t}") for t in range(NCH)]

            # ---- pre-zeroed diagonal P tiles (paired heads: [128,1024]) ----
            # halves: h0 cols [0:512), h1 cols [512:1024). For a diagonal
            # block with offset r, cols [0:128r) of each half are always
            # zero; zero them once, never rewrite.
            p_diag = {}
            for r in (1, 2, 3):
                for j in (0, 1):
                    pt = seq.tile([128, 2 * TCH], f32r, name=f"pdiag{r}_{j}")
                    pt3 = pt[:].rearrange("p (a q) -> p a q", a=2)
                    nc.vector.memset(pt3[:, :, 0:128 * r].bitcast(f32), 0.0)
                    p_diag[(r, j)] = pt

            # per-(b,bq) denominator tiles in wide layout [16, 64]:
            # rows 8h+p hold den[q = 64p + c] for head h
            dst_tiles = {}
            den_w = {}
            for b in range(B):
                for bq in range(NQC):
                    den_w[(b, bq)] = seq.tile([16, 64], f32,
                                              name=f"denw{b}{bq}")

            def qkv_chunk_gen(t):
                """qkv projection + V transpose for one 512-token chunk.
                Part-outer: one PSUM accumulator live at a time."""
                xts = [work.tile([128, TCH], f32r, tag="xt", bufs=16,
                                 name=f"xts{t}_{i}") for i in range(8)]
                for cb in range(8):
                    nc.gpsimd.dma_start(
                        xts[cb][:],
                        xt[cb * 128:(cb + 1) * 128, t * TCH:(t + 1) * TCH])
                vt_tmp = None
                for part in range(3):
                    ps = psBig.tile([128, TCH], f32, tag="big",
                                    name=f"pqkv{t}_{part}")
                    for cb in range(8):
                        nc.tensor.matmul(
                            ps[:], w_sb[:, cb, part * F:(part + 1) * F],
                            xts[cb][:], start=(cb == 0), stop=(cb == 7))
                        if cb == 3:
                            yield
                    if part == 0:
                        nc.vector.tensor_scalar_add(qt_t[t][:], ps[:],
                                                    b_sb[:, 0:1])
                    elif part == 1:
                        nc.vector.tensor_scalar_add(kt_t[t][:], ps[:],
                                                    b_sb[:, 1:2])
                    else:
                        vt_tmp = work.tile([128, TCH], f32, tag="vt",
                                           name=f"vt{t}")
                        nc.vector.tensor_scalar_add(vt_tmp[:], ps[:],
                                                    b_sb[:, 2:3])
                    yield
                ptr = psS.tile([128, TCH], f32, tag="s", name=f"ptr{t}")
                for i in range(4):
                    nc.tensor.transpose(ptr[:, i * 128:(i + 1) * 128],
                                        vt_tmp[:, i * 128:(i + 1) * 128],
                                        eye_sb[:])
                    if i == 1:
                        yield
                ptr3 = ptr[:].rearrange("p (a k) -> p a k", k=128)
                t4 = t * 4
                nc.vector.tensor_copy(v65[:, t4:t4 + 4, 0:64],
                                      ptr3[:, :, 0:64])
                nc.vector.tensor_copy(v65[:, t4:t4 + 4, 65:129],
                                      ptr3[:, :, 64:128])
                yield

            def norm_proj_gen(b, bq):
                """Per-chunk softmax normalization + projection."""
                qchunk = b * NQC + bq
                last = (b == B - 1 and bq == NQC - 1)
                if not last:
                    rec_w = work.tile([16, 64], f32r, tag="rec",
                                      name=f"rec{b}{bq}")
                    nc.vector.reciprocal(rec_w[:], den_w[(b, bq)][:])
                for h in range(HPC):
                    hs = h * 64
                    rst = work.tile([1, TCH], f32r, tag="rst",
                                    name=f"rst{b}{bq}{h}")
                    if last:
                        # tail chunk: skip the DRAM-roundtrip wide layout;
                        # a direct 1-lane reciprocal is lower latency here
                        nc.vector.reciprocal(rst[:], dst_tiles[(b, bq, h)][:])
                    else:
                        rr = dpool.tile([1, TCH], f32r, tag="rr",
                                        name=f"rr{b}{bq}{h}")
                        nc.gpsimd.dma_start(
                            rr[:].rearrange("o (p c) -> (o p) c", c=64),
                            rec_w[8 * h:8 * h + 8, :])
                        nc.gpsimd.dma_start(rst[:], rr[:])
                    bcast = work.tile([128, TCH], f32r, tag="bcast",
                                      name=f"bcast{b}{bq}{h}")
                    nc.gpsimd.partition_broadcast(bcast[:], rst[:])
                    nc.vector.tensor_mul(yt_t[qchunk][hs:hs + 64, :],
                                         yt_t[qchunk][hs:hs + 64, :],
                                         bcast[hs:hs + 64, :])
                yield
                for ic in range(4):
                    tt = qchunk * 4 + ic
                    for cc in range(2):
                        pj = psBig.tile([128, TCH], f32, tag="big",
                                        name=f"pj{tt}_{cc}")
                        nc.tensor.matmul(
                            pj[:],
                            yt_t[qchunk][:, ic * 128:(ic + 1) * 128],
                            wp_sb[:, cc * TCH:(cc + 1) * TCH],
                            start=True, stop=True)
                        ost = work.tile([128, TCH], f32, tag="ost",
                                        name=f"ost{tt}_{cc}")
                        if (ic + cc) % 2 == 0:
                            nc.scalar.copy(ost[:], pj[:])
                        else:
                            nc.vector.tensor_copy(ost[:], pj[:])
                        nc.sync.dma_start(
                            out[tt * 128:(tt + 1) * 128,
                                cc * TCH:(cc + 1) * TCH], ost[:])
                        yield

            class Filler:
                def __init__(self):
                    self.gens = []

                def add(self, g):
                    self.gens.append(g)

                def step(self):
                    while self.gens:
                        try:
                            next(self.gens[0])
                            return
                        except StopIteration:
                            self.gens.pop(0)

                def drain(self):
                    while self.gens:
                        for _ in self.gens.pop(0):
                            pass

            def attn_pair(b, bq, bk, use_idx):
                """S for both heads into one [128,1024] psum tile + one exp.
                Returns the P tile (halves = heads)."""
                qchunk = b * NQC + bq
                kchunk = b * NQC + bk // 4
                kcol = (bk % 4) * 128
                s_ps = psS.tile([128, 2 * TCH], f32, tag="s",
                                name=f"s{b}{bq}{bk}")
                r = bk - 4 * bq
                # masked q-columns [0:128r) can be skipped entirely when the
                # remaining width stays >= 256 (fp32r full-rate threshold)
                trim = 128 * r if r in (1, 2) else 0
                for h in range(HPC):
                    hs = h * 64
                    nc.tensor.matmul(
                        s_ps[:, h * TCH + trim:(h + 1) * TCH],
                        kt_t[kchunk][hs:hs + 64, kcol:kcol + 128],
                        qt_t[qchunk][hs:hs + 64, trim:],
                        start=True, stop=True)
                if r < 0:
                    p_t = work.tile([128, 2 * TCH], f32r, tag="p", bufs=4,
                                    name=f"p{b}{bq}{bk}")
                    nc.scalar.activation(p_t[:], s_ps[:], Exp)
                    return p_t
                if r == 0:
                    p_t = work.tile([128, 2 * TCH], f32r, tag="p", bufs=4,
                                    name=f"p{b}{bq}{bk}")
                    nc.scalar.activation(p_t[:], s_ps[:], Exp)
                else:
                    p_t = p_diag[(r, use_idx % 2)]
                    s3 = s_ps[:].rearrange("p (a q) -> p a q", a=2)
                    p3 = p_t[:].rearrange("p (a q) -> p a q", a=2)
                    nc.scalar.activation(p3[:, :, 128 * r:],
                                         s3[:, :, 128 * r:], Exp)
                for h in range(HPC):
                    c0 = h * TCH + 128 * r
                    nc.vector.tensor_mul(p_t[:, c0:c0 + 128],
                                         p_t[:, c0:c0 + 128], tri_sb[:])
                return p_t

            def attention_chunk(b, bq, fl):
                qchunk = b * NQC + bq
                nblk = 4 * bq + 4
                yt_ps = [psY.tile([65, TCH], f32, tag="yt",
                                  name=f"ytps{b}{bq}{h}")
                         for h in range(HPC)]
                pend = None

                def emit_pv(bk, p_t, stop):
                    vti = b * (T // KB) + bk
                    r = bk - 4 * bq
                    trim = 128 * r if r in (1, 2) else 0
                    for h in range(HPC):
                        nc.tensor.matmul(
                            yt_ps[h][:, trim:], v65[:, vti, 65 * h:65 * h + 65],
                            p_t[:, h * TCH + trim:(h + 1) * TCH],
                            start=(bk == 0), stop=stop)

                for bk in range(nblk):
                    p_t = attn_pair(b, bq, bk, bq)
                    if pend is not None:
                        emit_pv(pend[0], pend[1], stop=False)
                    pend = (bk, p_t)
                    fl.step()
                emit_pv(pend[0], pend[1], stop=True)
                for h in range(HPC):
                    hs = h * 64
                    nc.vector.tensor_copy(yt_t[qchunk][hs:hs + 64, :],
                                          yt_ps[h][0:64, :])
                    dst = work.tile([1, TCH], f32, tag="dst",
                                    name=f"dst{b}{bq}{h}")
                    nc.vector.tensor_copy(dst[:], yt_ps[h][64:65, :])
                    dst_tiles[(b, bq, h)] = dst
                    dd = dpool.tile([1, TCH], f32, tag="dd",
                                    name=f"dd{b}{bq}{h}")
                    nc.gpsimd.dma_start(dd[:], dst[:])
                    nc.gpsimd.dma_start(
                        den_w[(b, bq)][8 * h:8 * h + 8, :],
                        dd[:].rearrange("o (p c) -> (o p) c", c=64))
                fl.step()

            def chain(*gens):
                for g in gens:
                    yield from g

            # ---- schedule ----
            for t in range(NQC):
                for _ in qkv_chunk_gen(t):
                    pass
            fl = Filler()
            fl.add(chain(*[qkv_chunk_gen(t) for t in range(NQC, NCH)]))
            pending_np = []
            for b in range(B):
                for bq in range(NQC):
                    if b == B - 1 and bq == NQC - 1:
                        # last chunk: make all pending norm+proj work
                        # available as filler so only the final chunk's
                        # chain remains at the tail
                        while pending_np:
                            fl.add(pending_np.pop(0))
                    attention_chunk(b, bq, fl)
                    if pending_np:
                        fl.add(pending_np.pop(0))
                    pending_np.append(norm_proj_gen(b, bq))
            for g in pending_np:
                fl.add(g)
            fl.drain()
    nc.compile()
    return nc


def _get_nc():
    if "nc" not in _COMPILED:
        _COMPILED["nc"] = _build()
    return _COMPILED["nc"]


def _prep_in_maps(x, w_attn, b_attn, w_proj):
    x = np.asarray(x, np.float32)
    w_attn = np.asarray(w_attn, np.float32)
    b_attn = np.asarray(b_attn, np.float32)
    w_proj = np.asarray(w_proj, np.float32)

    scale = np.float32(1.0 / np.sqrt(D))
    xt = np.ascontiguousarray(x.reshape(BT, C).T)          # [C, BT]
    # tri[kv, j] = 1 when j >= kv (upper triangular incl diagonal)
    tri = np.ascontiguousarray(np.triu(np.ones((KB, KB), np.float32)))
    eye = np.eye(128, dtype=np.float32)
    ones = np.ones((128, 64), np.float32)

    in_maps = []
    for c in range(N_CORES):
        cols = slice(c * F, (c + 1) * F)
        wq = w_attn[:, cols] * scale
        wk = w_attn[:, C + c * F:C + (c + 1) * F]
        wv = w_attn[:, 2 * C + c * F:2 * C + (c + 1) * F]
        wqkv = np.ascontiguousarray(
            np.concatenate([wq, wk, wv], axis=1).reshape(C // 128, 128, 3 * F))
        bq = b_attn[c * F:(c + 1) * F] * scale
        bk = b_attn[C + c * F:C + (c + 1) * F]
        bv = b_attn[2 * C + c * F:2 * C + (c + 1) * F]
        bqkv = np.ascontiguousarray(np.stack([bq, bk, bv], axis=1))
        wp = np.ascontiguousarray(w_proj[c * F:(c + 1) * F, :])
        in_maps.append({
            "xt": xt, "wqkv": wqkv, "bqkv": bqkv, "wp": wp,
            "tri": tri, "eye": eye, "ones": ones,
        })
    return in_maps


def _run(inputs, trace=False):
    from concourse.bass_utils import run_bass_kernel_spmd

    nc = _get_nc()
    in_maps = _prep_in_maps(inputs["x"], inputs["w_attn"], inputs["b_attn"],
                            inputs["w_proj"])
    res = run_bass_kernel_spmd(nc, in_maps, list(range(N_CORES)), trace=trace)
    b_proj = np.asarray(inputs["b_proj"], np.float32)
    acc = np.zeros((BT, C), np.float64)
    for c in range(N_CORES):
        acc += res.results[c]["out"]
    y = (acc + b_proj).astype(np.float32).reshape(B, T, C)
    return y, res


def kernel(**inputs):
    y, _ = _run(inputs, trace=False)
    return y
